# revision 27
# baseline (speedup 1.0000x reference)
"""Trainium2 Bass kernel for AutoRegressiveGraphConvLayer.

Structure exploited (indices are compile-time constants):
  - ET (edge gather) is the identity permutation.
  - Node i's incoming edges are the contiguous block [k_i, k_i + min(i,32)).
  - Node-branch aggregation = per-node block sum of hn  (segment sum).
  - Edge-branch aggregation = exclusive prefix sum of he within each block.
  - Source-node gather for edge (i,j) is a sliding window over node index,
    expressible as an overlapping SBUF access pattern (no real gather).

Sharding: 8 cores = 2 batches x 4 contiguous node-range shards (256 nodes,
8192 padded edge slots each).  Every core runs the SAME program; the host
pre-pads node/edge slabs (prologue nodes i<32 get zero-padded window slots)
so the device code is fully uniform.

Layout: feature-major activations ([feat, token] on SBUF), weights stationary,
bf16 matmuls (fp32 psum).  First layers read the sliding-window / broadcast
node APs directly as matmul operands.  Second layers are computed token-major
(activations as the stationary operand) so outputs land pre-transposed for the
store.  Node aggregation = tiny matmuls against a per-subtile matrix A with
pad-mask and 1/deg norms folded in, accumulating into a persistent psum bank;
edge prefix sums = matmul with block-strict-lower-triangular T (norms folded).
Every per-tile psum tensor has its own single-buffered bank (8 total) so
cross-tile coupling only binds same-pipeline-position instructions.
"""

import numpy as np

MM_BF16 = True   # matmul precision: True -> bf16 (fast), False -> fp32

# problem constants
N, M = 1024, 32
FN, FE = 64, 32
AGG_N, AGG_E = 128, 64
OUT_N, OUT_E = 64, 32
NE = 32240
B = 2
NQ = 4              # node-range shards per batch
NSH = N // NQ       # 256 nodes per shard
ES = NSH * M        # 8192 padded edge slots per shard
TS = 512            # slots per tile
NT = ES // TS       # 16 tiles
NCORES = B * NQ

_cache = {}


def _tmin(i):
    return min(i, M)


def _koff(i):
    if i <= M:
        return i * (i - 1) // 2
    return M * (M - 1) // 2 + (i - M) * M


def _host_tables():
    if "tables" in _cache:
        return _cache["tables"]
    valid = []
    for i in range(NSH):
        t = _tmin(i)
        valid.extend(range(32 * i + (32 - t), 32 * i + 32))
    valid = np.array(valid, np.int64)
    assert valid.shape[0] == _koff(NSH)

    norm_node = np.full(N, 1.0 / M, np.float32)
    for i in range(1, M):
        norm_node[i] = 1.0 / i

    # T matrices [128, 9*128]: 8 prologue subtiles + 1 uniform.
    T_buf = np.zeros((128, 9 * 128), np.float32)
    for g in range(9):
        Tm = np.zeros((128, 128), np.float32)
        for m in range(128):
            blk, s = m // 32, m % 32
            t_i = 32 if g == 8 else _tmin(4 * g + blk)
            j = s - (32 - t_i)
            if j < 0:
                continue
            norm = 1.0 / j if j >= 1 else 1.0
            k0 = blk * 32 + (32 - t_i)
            k1 = blk * 32 + s
            if k1 > k0:
                Tm[k0:k1, m] = norm
        T_buf[:, g * 128:(g + 1) * 128] = Tm

    # A matrices [128, 9*4]: per subtile-group g, col c = node 4g+c (g<8
    # prologue) or any uniform node (g=8): norm on valid slots, 0 on pads
    A_buf = np.zeros((128, 36), np.float32)
    for g in range(9):
        for c in range(4):
            if g == 8:
                t_i, nm = 32, 1.0 / 32
            else:
                i = 4 * g + c
                t_i, nm = _tmin(i), norm_node[i]
            A_buf[32 * c + (32 - t_i):32 * c + 32, 4 * g + c] = nm

    _cache["tables"] = (valid, norm_node, T_buf, A_buf)
    return _cache["tables"]


def _build_program(zero_bias=True):
    key = ("prog", zero_bias)
    if key in _cache:
        return _cache[key]
    from contextlib import ExitStack

    import concourse.bass as bass
    import concourse.mybir as mybir
    import concourse.tile as tile
    from concourse import bacc
    from concourse.masks import make_identity

    dt = mybir.dt
    f32 = dt.float32
    mdt = dt.bfloat16 if MM_BF16 else f32
    Relu = mybir.ActivationFunctionType.Relu

    nc = bacc.Bacc("TRN2", target_bir_lowering=False, debug=True)

    def din(name, shape, d=f32):
        return nc.declare_dram_parameter(name, list(shape), d, isOutput=False)

    nodes_in = din("nodes_in", (NSH + 32, FN))
    edges_in = din("edges_in", (ES, FE))
    wshapes = [("w_an1_96", (96, 256)), ("w_an1I", (64, 256)),
               ("w_an2_lo", (128, 128)), ("w_an2_hi", (128, 128)),
               ("w_ae1", (96, 128)), ("w_ae2", (128, 64)),
               ("w_le1", (96, 96)), ("w_le2", (96, 32)),
               ("w_ln1_lo", (128, 192)), ("w_ln1_hi", (64, 192)),
               ("w_ln2_lo", (128, 64)), ("w_ln2_hi", (64, 64)),
               ("t_in", (128, 1152)), ("a_in", (128, 36)),
               ("b_an2r", (1, 128)), ("b_ae2r", (1, 64)), ("b_le2r", (1, 32))]
    bshapes = [("b_an1a", (128, 1)), ("b_an1b", (128, 1)), ("b_an2", (128, 1)),
               ("b_ae1", (128, 1)), ("b_ae2", (64, 1)), ("b_le1", (96, 1)),
               ("b_le2", (32, 1)), ("b_ln1a", (128, 1)), ("b_ln1b", (64, 1)),
               ("b_ln2", (64, 1))]
    wparams = {nm: din(nm, shp, mdt) for nm, shp in wshapes}
    bparams = {nm: din(nm, shp, f32) for nm, shp in bshapes}
    out_nodes = nc.declare_dram_parameter("out_nodes", [NSH, OUT_N], f32, isOutput=True)
    out_edges = nc.declare_dram_parameter("out_edges", [ES, OUT_E], f32, isOutput=True)

    def fap(base, pairs):
        """AP with base's tensor/partition dim/offset but custom free dims."""
        return bass.AP(tensor=base.tensor, offset=base.offset,
                       ap=[base.ap[0]] + [list(p) for p in pairs])

    with tile.TileContext(nc) as tc, ExitStack() as ctx:
        const = ctx.enter_context(tc.tile_pool(name="const", bufs=1))
        work = ctx.enter_context(tc.tile_pool(name="work", bufs=3))
        small = ctx.enter_context(tc.tile_pool(name="small", bufs=4))
        ps = ctx.enter_context(tc.tile_pool(name="ps", bufs=1, space="PSUM"))

        ident = const.tile([128, 128], f32)
        make_identity(nc, ident[:])

        def relu(out, in_, bias, eng="act"):
            # biases are all zero in this model instance -> plain max(x, 0)
            # on the chosen engine; fall back to ACT's fused bias otherwise
            if (zero_bias or bias is None) and eng == "dve":
                nc.vector.tensor_scalar_max(out, in_, 0.0)
            elif zero_bias or bias is None:
                nc.scalar.activation(out, in_, Relu)
            else:
                nc.scalar.activation(out, in_, Relu, bias=bias)

        W = {}
        for nm, shp in wshapes:
            tl = const.tile(list(shp), mdt, tag=nm)
            nc.scalar.dma_start(tl[:], wparams[nm][:])
            W[nm] = tl
        for nm, shp in bshapes:
            tl = const.tile(list(shp), f32, tag=nm)
            nc.sync.dma_start(tl[:], bparams[nm][:])
            W[nm] = tl

        # nodesT [64, 288] feature-major (mdt) via PE transposes
        nodesT = const.tile([FN, NSH + 32], mdt)
        for r0, rows in [(0, 128), (128, 128), (256, 32)]:
            stg = small.tile([128, FN], f32, tag="nstg")
            nc.sync.dma_start(stg[:rows, :], nodes_in[r0:r0 + rows, :])
            pt = ps.tile([FN, 128], f32, tag="pte")
            nc.tensor.transpose(pt[:, :rows], stg[:rows, :], ident[:rows, :rows])
            nc.vector.tensor_copy(nodesT[:, r0:r0 + rows], pt[:, :rows])

        pn = const.tile([AGG_N, NSH], mdt)
        ones = const.tile([1, 128], mdt)
        nc.vector.memset(ones[:], 1.0)
        # persistent psum bank: cols 0:256 accumulate pn^T (norm+mask folded
        # into a_in); cols 256:384 recycled per-tile for token-major out-edges
        pnpo = ps.tile([128, TS], f32, tag="pnpo")

        for t in range(NT):
            i0 = 16 * t
            c0 = TS * t

            # edges feature-major: 4 PE transposes into one psum tile
            stg = small.tile([128, 4 * FE], f32, tag="estg")
            nc.sync.dma_start(
                stg[:].rearrange("p (s f) -> p s f", s=4),
                edges_in[c0:c0 + TS, :].rearrange("(s p) f -> p s f", p=128))
            pte = ps.tile([FE, TS], f32, tag="pte")
            for s in range(4):
                nc.tensor.transpose(pte[:, 128 * s:128 * (s + 1)],
                                    stg[:, FE * s:FE * (s + 1)], ident[:])

            # sj [96, 512]: rows 0:64 src-node sliding window, rows 64:96
            # edges feature-major; dst-node contribution via broadcast AP
            sj = work.tile([96, TS], mdt)
            srcw = fap(nodesT[0:FN, i0:i0 + 1], [[1, 16], [1, 32]])
            nc.gpsimd.tensor_copy(sj[0:64, :].rearrange("p (a b) -> p a b", a=16), srcw)
            nc.vector.tensor_copy(sj[64:96, :], pte[:])
            dstb = fap(nodesT[0:FN, 32 + i0:33 + i0], [[1, 16], [0, 32]])

            # node branch layer 1
            pa = ps.tile([128, TS], f32, tag="pab")
            nc.tensor.matmul(pa[:], W["w_an1_96"][:, 0:128], sj[:], start=True, stop=False)
            nc.tensor.matmul(pa[:], W["w_an1I"][:, 0:128], dstb, start=False, stop=True)
            pb = ps.tile([128, TS], f32, tag="pab")
            nc.tensor.matmul(pb[:], W["w_an1_96"][:, 128:256], sj[:], start=True, stop=False)
            nc.tensor.matmul(pb[:], W["w_an1I"][:, 128:256], dstb, start=False, stop=True)
            h1na = work.tile([128, TS], mdt)
            relu(h1na[:], pa[:], W["b_an1a"][:, 0:1], "act")
            h1nb = work.tile([128, TS], mdt)
            relu(h1nb[:], pb[:], W["b_an1b"][:, 0:1], "act")

            # node branch layer 2, token-major: hn_tok [tok, feat] per subtile
            phnT = ps.tile([128, TS], f32, tag="phnT")
            for s in range(4):
                sl = slice(128 * s, 128 * (s + 1))
                if not zero_bias:
                    nc.tensor.matmul(phnT[:, sl], ones[:], W["b_an2r"][:],
                                     start=True, stop=False)
                nc.tensor.matmul(phnT[:, sl], h1na[:, sl], W["w_an2_lo"][:],
                                 start=zero_bias, stop=False)
                nc.tensor.matmul(phnT[:, sl], h1nb[:, sl], W["w_an2_hi"][:],
                                 start=False, stop=True)
            hn_tok = work.tile([128, TS], mdt)
            relu(hn_tok[:], phnT[:], None, "act")

            # node aggregation: tiny matmuls with block-mask+norm matrix A,
            # accumulating pn^T in the persistent psum bank
            for s in range(4):
                g = min(4 * t + s, 8)
                st = 4 * t + s
                nc.tensor.matmul(pnpo[:, 4 * st:4 * st + 4],
                                 hn_tok[:, 128 * s:128 * (s + 1)],
                                 W["a_in"][:, 4 * g:4 * g + 4],
                                 start=True, stop=True)

            # edge branch layer 1
            pe1 = ps.tile([128, TS], f32, tag="pe1")
            nc.tensor.matmul(pe1[:], W["w_ae1"][:], sj[:], start=True, stop=True)
            h1e = work.tile([128, TS], mdt)
            relu(h1e[:], pe1[:], W["b_ae1"][:, 0:1], "act")

            # edge branch layer 2, token-major -> he_tok directly
            pheT = ps.tile([128, 4 * AGG_E], f32, tag="pheT")
            for s in range(4):
                sl = slice(AGG_E * s, AGG_E * (s + 1))
                if not zero_bias:
                    nc.tensor.matmul(pheT[:, sl], ones[:], W["b_ae2r"][:],
                                     start=True, stop=False)
                nc.tensor.matmul(pheT[:, sl], h1e[:, 128 * s:128 * (s + 1)],
                                 W["w_ae2"][:], start=zero_bias, stop=True)
            he_tok = small.tile([128, 4 * AGG_E], mdt, tag="hetok")
            relu(he_tok[:], pheT[:], None, "dve")

            # blockwise exclusive prefix sums (norms folded into T)
            ppe = ps.tile([AGG_E, TS], f32, tag="ppe")
            for s in range(4):
                g = min(4 * t + s, 8)
                nc.tensor.matmul(ppe[:, 128 * s:128 * (s + 1)],
                                 he_tok[:, AGG_E * s:AGG_E * (s + 1)],
                                 W["t_in"][:, 128 * g:128 * (g + 1)],
                                 start=True, stop=True)

            # out-edge MLP: K-split, edge part read from sj at equal base
            pe_sb = work.tile([AGG_E, TS], mdt)
            nc.vector.tensor_copy(pe_sb[:], ppe[:])
            po1 = ps.tile([96, TS], f32, tag="po1")
            nc.tensor.matmul(po1[:], W["w_le1"][0:64, :], pe_sb[:], start=True, stop=False)
            nc.tensor.matmul(po1[:], W["w_le1"][64:96, :], sj[64:96, :], start=False, stop=True)
            h2e = work.tile([96, TS], mdt)
            relu(h2e[:], po1[:], W["b_le1"][:, 0:1], "dve")

            # out-edge layer 2, token-major, into cols 256:384 of pnpo bank
            for s in range(4):
                sl = slice(256 + OUT_E * s, 256 + OUT_E * (s + 1))
                if not zero_bias:
                    nc.tensor.matmul(pnpo[:, sl], ones[:], W["b_le2r"][:],
                                     start=True, stop=False)
                nc.tensor.matmul(pnpo[:, sl], h2e[:, 128 * s:128 * (s + 1)],
                                 W["w_le2"][:], start=zero_bias, stop=True)
            oe_tok = small.tile([128, 4 * OUT_E], f32, tag="oetok")
            relu(oe_tok[:], pnpo[:, 256:384], None, "dve")
            nc.scalar.dma_start(
                out_edges[c0:c0 + TS, :].rearrange("(s p) f -> p s f", p=128),
                oe_tok[:].rearrange("p (s f) -> p s f", s=4))

        # ---- node-output MLP ----
        nc.vector.tensor_copy(pn[:], pnpo[:, 0:NSH])
        nrhs = nodesT[:, 32:32 + NSH]
        pn1a = ps.tile([128, NSH], f32, tag="pab")
        nc.tensor.matmul(pn1a[:], W["w_ln1_lo"][:, 0:128], pn[:], start=True, stop=False)
        nc.tensor.matmul(pn1a[:], W["w_ln1_hi"][:, 0:128], nrhs, start=False, stop=True)
        pn1b = ps.tile([64, NSH], f32, tag="pab")
        nc.tensor.matmul(pn1b[:], W["w_ln1_lo"][:, 128:192], pn[:], start=True, stop=False)
        nc.tensor.matmul(pn1b[:], W["w_ln1_hi"][:, 128:192], nrhs, start=False, stop=True)
        h2na = work.tile([128, NSH], mdt)
        nc.scalar.activation(h2na[:], pn1a[:], Relu, bias=W["b_ln1a"][:, 0:1])
        h2nb = work.tile([64, NSH], mdt)
        nc.scalar.activation(h2nb[:], pn1b[:], Relu, bias=W["b_ln1b"][:, 0:1])
        pn2 = ps.tile([OUT_N, NSH], f32, tag="pe1")
        nc.tensor.matmul(pn2[:], W["w_ln2_lo"][:], h2na[:], start=True, stop=False)
        nc.tensor.matmul(pn2[:], W["w_ln2_hi"][:], h2nb[:], start=False, stop=True)
        on_fm = work.tile([OUT_N, NSH], f32)
        nc.scalar.activation(on_fm[:], pn2[:], Relu, bias=W["b_ln2"][:, 0:1])
        for s in range(2):
            pnt = ps.tile([128, OUT_N], f32, tag="pheT")
            nc.tensor.transpose(pnt[:], on_fm[:, 128 * s:128 * (s + 1)], ident[0:64, 0:64])
            on_tok = small.tile([128, OUT_N], f32, tag="ontok")
            nc.vector.tensor_copy(on_tok[:], pnt[:])
            nc.sync.dma_start(out_nodes[128 * s:128 * (s + 1), :], on_tok[:])

    nc.compile()
    _cache[key] = nc
    return nc


def _wcast(a):
    if MM_BF16:
        import ml_dtypes
        return np.ascontiguousarray(a.astype(ml_dtypes.bfloat16))
    return np.ascontiguousarray(a, np.float32)


def _prep_core_inputs(inputs, b, q):
    valid, norm_node, T_buf, A_buf = _host_tables()
    f32 = np.float32
    nodes = np.asarray(inputs["input_nodes"], f32)[b]
    edges = np.asarray(inputs["input_edges"], f32)[b]
    n0 = q * NSH
    if q == 0:
        nslab = np.concatenate([np.zeros((32, FN), f32), nodes[0:NSH]], axis=0)
        eslab = np.zeros((ES, FE), f32)
        for i in range(1, 32):
            k = _koff(i)
            eslab[32 * i + (32 - i):32 * (i + 1)] = edges[k:k + i]
        eslab[1024:] = edges[_koff(32):_koff(NSH)]
        tb = T_buf
        ab = A_buf
    else:
        nslab = nodes[n0 - 32:n0 + NSH]
        eslab = edges[_koff(n0):_koff(n0) + ES]
        tb = np.tile(T_buf[:, 8 * 128:9 * 128], (1, 9))
        ab = np.tile(A_buf[:, 32:36], (1, 9))

    Wan1 = np.asarray(inputs["Wan1"], f32)
    Wan2 = np.asarray(inputs["Wan2"], f32)
    Wln1 = np.asarray(inputs["Wln1"], f32)
    Wln2 = np.asarray(inputs["Wln2"], f32)
    m = {
        "nodes_in": np.ascontiguousarray(nslab, f32),
        "edges_in": np.ascontiguousarray(eslab, f32),
        "w_an1_96": _wcast(Wan1[0:96]), "w_an1I": _wcast(Wan1[96:160]),
        "w_an2_lo": _wcast(Wan2[0:128]), "w_an2_hi": _wcast(Wan2[128:256]),
        "w_ae1": _wcast(np.asarray(inputs["Wae1"], f32)),
        "w_le1": _wcast(np.asarray(inputs["Wle1"], f32)),
        "w_ae2": _wcast(np.asarray(inputs["Wae2"], f32)),
        "w_le2": _wcast(np.asarray(inputs["Wle2"], f32)),
        "w_ln1_lo": _wcast(Wln1[0:128]), "w_ln1_hi": _wcast(Wln1[128:192]),
        "w_ln2_lo": _wcast(Wln2[0:128]), "w_ln2_hi": _wcast(Wln2[128:192]),
        "t_in": _wcast(tb),
        "a_in": _wcast(ab),
        "b_an2r": _wcast(np.asarray(inputs["ban2"], f32).reshape(1, 128)),
        "b_ae2r": _wcast(np.asarray(inputs["bae2"], f32).reshape(1, 64)),
        "b_le2r": _wcast(np.asarray(inputs["ble2"], f32).reshape(1, 32)),
        "b_an1a": np.ascontiguousarray(np.asarray(inputs["ban1"], f32)[0:128].reshape(128, 1)),
        "b_an1b": np.ascontiguousarray(np.asarray(inputs["ban1"], f32)[128:256].reshape(128, 1)),
        "b_an2": np.ascontiguousarray(np.asarray(inputs["ban2"], f32).reshape(AGG_N, 1)),
        "b_ae1": np.ascontiguousarray(np.asarray(inputs["bae1"], f32).reshape(128, 1)),
        "b_ae2": np.ascontiguousarray(np.asarray(inputs["bae2"], f32).reshape(AGG_E, 1)),
        "b_le1": np.ascontiguousarray(np.asarray(inputs["ble1"], f32).reshape(96, 1)),
        "b_le2": np.ascontiguousarray(np.asarray(inputs["ble2"], f32).reshape(OUT_E, 1)),
        "b_ln1a": np.ascontiguousarray(np.asarray(inputs["bln1"], f32)[0:128].reshape(128, 1)),
        "b_ln1b": np.ascontiguousarray(np.asarray(inputs["bln1"], f32)[128:192].reshape(64, 1)),
        "b_ln2": np.ascontiguousarray(np.asarray(inputs["bln2"], f32).reshape(OUT_N, 1)),
    }
    return m


def kernel(**inputs):
    import sys
    if "/opt/trn_rl_repo" not in sys.path:
        sys.path.insert(0, "/opt/trn_rl_repo")
    from concourse.bass_utils import run_bass_kernel_spmd

    zb = all(not np.any(np.asarray(inputs[k])) for k in
             ("ban1", "ban2", "bae1", "bae2", "ble1", "ble2", "bln1", "bln2"))
    nc = _build_program(zero_bias=zb)
    in_maps = []
    for c in range(NCORES):
        b, q = c // NQ, c % NQ
        in_maps.append(_prep_core_inputs(inputs, b, q))
    res = run_bass_kernel_spmd(nc, in_maps, list(range(NCORES))).results

    valid = _host_tables()[0]
    out_nodes = np.zeros((B, N, OUT_N), np.float32)
    out_edges = np.zeros((B, NE, OUT_E), np.float32)
    for c in range(NCORES):
        b, q = c // NQ, c % NQ
        n0 = q * NSH
        out_nodes[b, n0:n0 + NSH] = res[c]["out_nodes"]
        if q == 0:
            out_edges[b, 0:_koff(NSH)] = res[c]["out_edges"][valid]
        else:
            out_edges[b, _koff(n0):_koff(n0) + ES] = res[c]["out_edges"]
    return out_nodes, out_edges


# revision 29
# speedup vs baseline: 1.0396x; 1.0396x over previous
"""Trainium2 Bass kernel for AutoRegressiveGraphConvLayer.

Structure exploited (indices are compile-time constants):
  - ET (edge gather) is the identity permutation.
  - Node i's incoming edges are the contiguous block [k_i, k_i + min(i,32)).
  - Node-branch aggregation = per-node block sum of hn  (segment sum).
  - Edge-branch aggregation = exclusive prefix sum of he within each block.
  - Source-node gather for edge (i,j) is a sliding window over node index,
    expressible as an overlapping SBUF access pattern (no real gather).

Sharding: 8 cores = 2 batches x 4 contiguous node-range shards (256 nodes,
8192 padded edge slots each).  Every core runs the SAME program; the host
pre-pads node/edge slabs (prologue nodes i<32 get zero-padded window slots)
so the device code is fully uniform.

Layout: feature-major activations ([feat, token] on SBUF), weights stationary,
bf16 matmuls (fp32 psum).  First layers read the sliding-window / broadcast
node APs directly as matmul operands.  Second layers are computed token-major
(activations as the stationary operand) so outputs land pre-transposed for the
store.  Node aggregation = tiny matmuls against a per-subtile matrix A with
pad-mask and 1/deg norms folded in, accumulating into a persistent psum bank;
edge prefix sums = matmul with block-strict-lower-triangular T (norms folded).
Every per-tile psum tensor has its own single-buffered bank (8 total) so
cross-tile coupling only binds same-pipeline-position instructions.
"""

import numpy as np

MM_BF16 = True   # matmul precision: True -> bf16 (fast), False -> fp32

# problem constants
N, M = 1024, 32
FN, FE = 64, 32
AGG_N, AGG_E = 128, 64
OUT_N, OUT_E = 64, 32
NE = 32240
B = 2
NQ = 4              # node-range shards per batch
NSH = N // NQ       # 256 nodes per shard
ES = NSH * M        # 8192 padded edge slots per shard
TS = 512            # slots per tile
NT = ES // TS       # 16 tiles
NCORES = B * NQ

_cache = {}


def _tmin(i):
    return min(i, M)


def _koff(i):
    if i <= M:
        return i * (i - 1) // 2
    return M * (M - 1) // 2 + (i - M) * M


def _host_tables():
    if "tables" in _cache:
        return _cache["tables"]
    valid = []
    for i in range(NSH):
        t = _tmin(i)
        valid.extend(range(32 * i + (32 - t), 32 * i + 32))
    valid = np.array(valid, np.int64)
    assert valid.shape[0] == _koff(NSH)

    norm_node = np.full(N, 1.0 / M, np.float32)
    for i in range(1, M):
        norm_node[i] = 1.0 / i

    # T matrices [128, 9*128]: 8 prologue subtiles + 1 uniform.
    T_buf = np.zeros((128, 9 * 128), np.float32)
    for g in range(9):
        Tm = np.zeros((128, 128), np.float32)
        for m in range(128):
            blk, s = m // 32, m % 32
            t_i = 32 if g == 8 else _tmin(4 * g + blk)
            j = s - (32 - t_i)
            if j < 0:
                continue
            norm = 1.0 / j if j >= 1 else 1.0
            k0 = blk * 32 + (32 - t_i)
            k1 = blk * 32 + s
            if k1 > k0:
                Tm[k0:k1, m] = norm
        T_buf[:, g * 128:(g + 1) * 128] = Tm

    # A matrices [128, 9*4]: per subtile-group g, col c = node 4g+c (g<8
    # prologue) or any uniform node (g=8): norm on valid slots, 0 on pads
    A_buf = np.zeros((128, 36), np.float32)
    for g in range(9):
        for c in range(4):
            if g == 8:
                t_i, nm = 32, 1.0 / 32
            else:
                i = 4 * g + c
                t_i, nm = _tmin(i), norm_node[i]
            A_buf[32 * c + (32 - t_i):32 * c + 32, 4 * g + c] = nm

    _cache["tables"] = (valid, norm_node, T_buf, A_buf)
    return _cache["tables"]


def _build_program(zero_bias=True):
    key = ("prog", zero_bias)
    if key in _cache:
        return _cache[key]
    from contextlib import ExitStack

    import concourse.bass as bass
    import concourse.mybir as mybir
    import concourse.tile as tile
    from concourse import bacc
    from concourse.masks import make_identity

    dt = mybir.dt
    f32 = dt.float32
    mdt = dt.bfloat16 if MM_BF16 else f32
    Relu = mybir.ActivationFunctionType.Relu

    nc = bacc.Bacc("TRN2", target_bir_lowering=False, debug=True)

    def din(name, shape, d=f32):
        return nc.declare_dram_parameter(name, list(shape), d, isOutput=False)

    nodes_in = din("nodes_in", (NSH + 32, FN))
    edges_in = din("edges_in", (ES, FE))
    wshapes = [("w_an1_96", (96, 256)), ("w_an1I", (64, 256)),
               ("w_an2_lo", (128, 128)), ("w_an2_hi", (128, 128)),
               ("w_ae1", (96, 128)), ("w_ae2", (128, 64)),
               ("w_le1", (96, 96)), ("w_le2", (96, 32)),
               ("w_ln1_lo", (128, 192)), ("w_ln1_hi", (64, 192)),
               ("w_ln2_lo", (128, 64)), ("w_ln2_hi", (64, 64)),
               ("t_in", (128, 1152)), ("a_in", (128, 36)),
               ("b_an2r", (1, 128)), ("b_ae2r", (1, 64)), ("b_le2r", (1, 32))]
    bshapes = [("b_an1a", (128, 1)), ("b_an1b", (128, 1)), ("b_an2", (128, 1)),
               ("b_ae1", (128, 1)), ("b_ae2", (64, 1)), ("b_le1", (96, 1)),
               ("b_le2", (32, 1)), ("b_ln1a", (128, 1)), ("b_ln1b", (64, 1)),
               ("b_ln2", (64, 1))]
    wparams = {nm: din(nm, shp, mdt) for nm, shp in wshapes}
    bparams = {nm: din(nm, shp, f32) for nm, shp in bshapes}
    out_nodes = nc.declare_dram_parameter("out_nodes", [NSH, OUT_N], f32, isOutput=True)
    out_edges = nc.declare_dram_parameter("out_edges", [ES, OUT_E], f32, isOutput=True)

    def fap(base, pairs):
        """AP with base's tensor/partition dim/offset but custom free dims."""
        return bass.AP(tensor=base.tensor, offset=base.offset,
                       ap=[base.ap[0]] + [list(p) for p in pairs])

    with tile.TileContext(nc) as tc, ExitStack() as ctx:
        const = ctx.enter_context(tc.tile_pool(name="const", bufs=1))
        work = ctx.enter_context(tc.tile_pool(name="work", bufs=3))
        small = ctx.enter_context(tc.tile_pool(name="small", bufs=4))
        ps = ctx.enter_context(tc.tile_pool(name="ps", bufs=1, space="PSUM"))

        ident = const.tile([128, 128], f32)
        make_identity(nc, ident[:])

        def relu(out, in_, bias, eng="act"):
            # biases are all zero in this model instance -> plain max(x, 0)
            # on the chosen engine; fall back to ACT's fused bias otherwise
            if (zero_bias or bias is None) and eng == "dve":
                nc.vector.tensor_scalar_max(out, in_, 0.0)
            elif zero_bias or bias is None:
                nc.scalar.activation(out, in_, Relu)
            else:
                nc.scalar.activation(out, in_, Relu, bias=bias)

        W = {}
        for nm, shp in wshapes:
            tl = const.tile(list(shp), mdt, tag=nm)
            nc.scalar.dma_start(tl[:], wparams[nm][:])
            W[nm] = tl
        for nm, shp in bshapes:
            if zero_bias:
                continue  # unread on device; skip the startup DMAs
            tl = const.tile(list(shp), f32, tag=nm)
            nc.sync.dma_start(tl[:], bparams[nm][:])
            W[nm] = tl

        def bias_ap(nm):
            return None if zero_bias else W[nm][:, 0:1]

        # nodesT [64, 288] feature-major (mdt) via PE transposes
        nodesT = const.tile([FN, NSH + 32], mdt)
        for r0, rows in [(0, 128), (128, 128), (256, 32)]:
            stg = small.tile([128, FN], f32, tag="nstg")
            nc.sync.dma_start(stg[:rows, :], nodes_in[r0:r0 + rows, :])
            pt = ps.tile([FN, 128], f32, tag="pte")
            nc.tensor.transpose(pt[:, :rows], stg[:rows, :], ident[:rows, :rows])
            nc.vector.tensor_copy(nodesT[:, r0:r0 + rows], pt[:, :rows])

        pn = const.tile([AGG_N, NSH], mdt)
        ones = const.tile([1, 128], mdt)
        nc.vector.memset(ones[:], 1.0)
        # persistent psum bank: cols 0:256 accumulate pn^T (norm+mask folded
        # into a_in); cols 256:384 recycled per-tile for token-major out-edges
        pnpo = ps.tile([128, TS], f32, tag="pnpo")

        for t in range(NT):
            i0 = 16 * t
            c0 = TS * t

            # edges feature-major: 4 PE transposes into one psum tile
            stg = small.tile([128, 4 * FE], f32, tag="estg")
            nc.sync.dma_start(
                stg[:].rearrange("p (s f) -> p s f", s=4),
                edges_in[c0:c0 + TS, :].rearrange("(s p) f -> p s f", p=128))
            pte = ps.tile([FE, TS], f32, tag="pte")
            for s in range(4):
                nc.tensor.transpose(pte[:, 128 * s:128 * (s + 1)],
                                    stg[:, FE * s:FE * (s + 1)], ident[:])

            # sj [96, 512]: rows 0:64 src-node sliding window, rows 64:96
            # edges feature-major; dst-node contribution via broadcast AP
            sj = work.tile([96, TS], mdt)
            srcw = fap(nodesT[0:FN, i0:i0 + 1], [[1, 16], [1, 32]])
            nc.gpsimd.tensor_copy(sj[0:64, :].rearrange("p (a b) -> p a b", a=16), srcw)
            nc.vector.tensor_copy(sj[64:96, :], pte[:])
            dstb = fap(nodesT[0:FN, 32 + i0:33 + i0], [[1, 16], [0, 32]])

            # node branch layer 1
            pa = ps.tile([128, TS], f32, tag="pab")
            nc.tensor.matmul(pa[:], W["w_an1_96"][:, 0:128], sj[:], start=True, stop=False)
            nc.tensor.matmul(pa[:], W["w_an1I"][:, 0:128], dstb, start=False, stop=True)
            pb = ps.tile([128, TS], f32, tag="pab")
            nc.tensor.matmul(pb[:], W["w_an1_96"][:, 128:256], sj[:], start=True, stop=False)
            nc.tensor.matmul(pb[:], W["w_an1I"][:, 128:256], dstb, start=False, stop=True)
            h1na = work.tile([128, TS], mdt)
            relu(h1na[:], pa[:], bias_ap("b_an1a"), "act")
            h1nb = work.tile([128, TS], mdt)
            relu(h1nb[:], pb[:], bias_ap("b_an1b"), "act")

            # node branch layer 2, token-major: hn_tok [tok, feat] per subtile
            phnT = ps.tile([128, TS], f32, tag="phnT")
            for s in range(4):
                sl = slice(128 * s, 128 * (s + 1))
                if not zero_bias:
                    nc.tensor.matmul(phnT[:, sl], ones[:], W["b_an2r"][:],
                                     start=True, stop=False)
                nc.tensor.matmul(phnT[:, sl], h1na[:, sl], W["w_an2_lo"][:],
                                 start=zero_bias, stop=False)
                nc.tensor.matmul(phnT[:, sl], h1nb[:, sl], W["w_an2_hi"][:],
                                 start=False, stop=True)
            hn_tok = work.tile([128, TS], mdt)
            relu(hn_tok[:], phnT[:], None, "act")

            # node aggregation: tiny matmuls with block-mask+norm matrix A,
            # accumulating pn^T in the persistent psum bank
            for s in range(4):
                g = min(4 * t + s, 8)
                st = 4 * t + s
                nc.tensor.matmul(pnpo[:, 4 * st:4 * st + 4],
                                 hn_tok[:, 128 * s:128 * (s + 1)],
                                 W["a_in"][:, 4 * g:4 * g + 4],
                                 start=True, stop=True)

            # edge branch layer 1
            pe1 = ps.tile([128, TS], f32, tag="pe1")
            nc.tensor.matmul(pe1[:], W["w_ae1"][:], sj[:], start=True, stop=True)
            h1e = work.tile([128, TS], mdt)
            relu(h1e[:], pe1[:], bias_ap("b_ae1"), "act")

            # edge branch layer 2, token-major -> he_tok directly
            pheT = ps.tile([128, 4 * AGG_E], f32, tag="pheT")
            for s in range(4):
                sl = slice(AGG_E * s, AGG_E * (s + 1))
                if not zero_bias:
                    nc.tensor.matmul(pheT[:, sl], ones[:], W["b_ae2r"][:],
                                     start=True, stop=False)
                nc.tensor.matmul(pheT[:, sl], h1e[:, 128 * s:128 * (s + 1)],
                                 W["w_ae2"][:], start=zero_bias, stop=True)
            he_tok = small.tile([128, 4 * AGG_E], mdt, tag="hetok")
            relu(he_tok[:], pheT[:], None, "dve")

            # blockwise exclusive prefix sums (norms folded into T)
            ppe = ps.tile([AGG_E, TS], f32, tag="ppe")
            for s in range(4):
                g = min(4 * t + s, 8)
                nc.tensor.matmul(ppe[:, 128 * s:128 * (s + 1)],
                                 he_tok[:, AGG_E * s:AGG_E * (s + 1)],
                                 W["t_in"][:, 128 * g:128 * (g + 1)],
                                 start=True, stop=True)

            # out-edge MLP: K-split, edge part read from sj at equal base
            pe_sb = work.tile([AGG_E, TS], mdt)
            nc.vector.tensor_copy(pe_sb[:], ppe[:])
            po1 = ps.tile([96, TS], f32, tag="po1")
            nc.tensor.matmul(po1[:], W["w_le1"][0:64, :], pe_sb[:], start=True, stop=False)
            nc.tensor.matmul(po1[:], W["w_le1"][64:96, :], sj[64:96, :], start=False, stop=True)
            h2e = work.tile([96, TS], mdt)
            relu(h2e[:], po1[:], bias_ap("b_le1"), "dve")

            # out-edge layer 2, token-major, into cols 256:384 of pnpo bank
            for s in range(4):
                sl = slice(256 + OUT_E * s, 256 + OUT_E * (s + 1))
                if not zero_bias:
                    nc.tensor.matmul(pnpo[:, sl], ones[:], W["b_le2r"][:],
                                     start=True, stop=False)
                nc.tensor.matmul(pnpo[:, sl], h2e[:, 128 * s:128 * (s + 1)],
                                 W["w_le2"][:], start=zero_bias, stop=True)
            oe_tok = small.tile([128, 4 * OUT_E], f32, tag="oetok")
            relu(oe_tok[:], pnpo[:, 256:384], None, "dve")
            nc.scalar.dma_start(
                out_edges[c0:c0 + TS, :].rearrange("(s p) f -> p s f", p=128),
                oe_tok[:].rearrange("p (s f) -> p s f", s=4))

        # ---- node-output MLP ----
        nc.vector.tensor_copy(pn[:], pnpo[:, 0:NSH])
        nrhs = nodesT[:, 32:32 + NSH]
        pn1a = ps.tile([128, NSH], f32, tag="pab")
        nc.tensor.matmul(pn1a[:], W["w_ln1_lo"][:, 0:128], pn[:], start=True, stop=False)
        nc.tensor.matmul(pn1a[:], W["w_ln1_hi"][:, 0:128], nrhs, start=False, stop=True)
        pn1b = ps.tile([64, NSH], f32, tag="pab")
        nc.tensor.matmul(pn1b[:], W["w_ln1_lo"][:, 128:192], pn[:], start=True, stop=False)
        nc.tensor.matmul(pn1b[:], W["w_ln1_hi"][:, 128:192], nrhs, start=False, stop=True)
        h2na = work.tile([128, NSH], mdt)
        relu(h2na[:], pn1a[:], bias_ap("b_ln1a"), "act")
        h2nb = work.tile([64, NSH], mdt)
        relu(h2nb[:], pn1b[:], bias_ap("b_ln1b"), "act")
        pn2 = ps.tile([OUT_N, NSH], f32, tag="pe1")
        nc.tensor.matmul(pn2[:], W["w_ln2_lo"][:], h2na[:], start=True, stop=False)
        nc.tensor.matmul(pn2[:], W["w_ln2_hi"][:], h2nb[:], start=False, stop=True)
        on_fm = work.tile([OUT_N, NSH], f32)
        relu(on_fm[:], pn2[:], bias_ap("b_ln2"), "act")
        for s in range(2):
            pnt = ps.tile([128, OUT_N], f32, tag="pheT")
            nc.tensor.transpose(pnt[:], on_fm[:, 128 * s:128 * (s + 1)], ident[0:64, 0:64])
            on_tok = small.tile([128, OUT_N], f32, tag="ontok")
            nc.vector.tensor_copy(on_tok[:], pnt[:])
            nc.sync.dma_start(out_nodes[128 * s:128 * (s + 1), :], on_tok[:])

    nc.compile()
    _cache[key] = nc
    return nc


def _wcast(a):
    if MM_BF16:
        import ml_dtypes
        return np.ascontiguousarray(a.astype(ml_dtypes.bfloat16))
    return np.ascontiguousarray(a, np.float32)


def _prep_core_inputs(inputs, b, q):
    valid, norm_node, T_buf, A_buf = _host_tables()
    f32 = np.float32
    nodes = np.asarray(inputs["input_nodes"], f32)[b]
    edges = np.asarray(inputs["input_edges"], f32)[b]
    n0 = q * NSH
    if q == 0:
        nslab = np.concatenate([np.zeros((32, FN), f32), nodes[0:NSH]], axis=0)
        eslab = np.zeros((ES, FE), f32)
        for i in range(1, 32):
            k = _koff(i)
            eslab[32 * i + (32 - i):32 * (i + 1)] = edges[k:k + i]
        eslab[1024:] = edges[_koff(32):_koff(NSH)]
        tb = T_buf
        ab = A_buf
    else:
        nslab = nodes[n0 - 32:n0 + NSH]
        eslab = edges[_koff(n0):_koff(n0) + ES]
        tb = np.tile(T_buf[:, 8 * 128:9 * 128], (1, 9))
        ab = np.tile(A_buf[:, 32:36], (1, 9))

    Wan1 = np.asarray(inputs["Wan1"], f32)
    Wan2 = np.asarray(inputs["Wan2"], f32)
    Wln1 = np.asarray(inputs["Wln1"], f32)
    Wln2 = np.asarray(inputs["Wln2"], f32)
    m = {
        "nodes_in": np.ascontiguousarray(nslab, f32),
        "edges_in": np.ascontiguousarray(eslab, f32),
        "w_an1_96": _wcast(Wan1[0:96]), "w_an1I": _wcast(Wan1[96:160]),
        "w_an2_lo": _wcast(Wan2[0:128]), "w_an2_hi": _wcast(Wan2[128:256]),
        "w_ae1": _wcast(np.asarray(inputs["Wae1"], f32)),
        "w_le1": _wcast(np.asarray(inputs["Wle1"], f32)),
        "w_ae2": _wcast(np.asarray(inputs["Wae2"], f32)),
        "w_le2": _wcast(np.asarray(inputs["Wle2"], f32)),
        "w_ln1_lo": _wcast(Wln1[0:128]), "w_ln1_hi": _wcast(Wln1[128:192]),
        "w_ln2_lo": _wcast(Wln2[0:128]), "w_ln2_hi": _wcast(Wln2[128:192]),
        "t_in": _wcast(tb),
        "a_in": _wcast(ab),
        "b_an2r": _wcast(np.asarray(inputs["ban2"], f32).reshape(1, 128)),
        "b_ae2r": _wcast(np.asarray(inputs["bae2"], f32).reshape(1, 64)),
        "b_le2r": _wcast(np.asarray(inputs["ble2"], f32).reshape(1, 32)),
        "b_an1a": np.ascontiguousarray(np.asarray(inputs["ban1"], f32)[0:128].reshape(128, 1)),
        "b_an1b": np.ascontiguousarray(np.asarray(inputs["ban1"], f32)[128:256].reshape(128, 1)),
        "b_an2": np.ascontiguousarray(np.asarray(inputs["ban2"], f32).reshape(AGG_N, 1)),
        "b_ae1": np.ascontiguousarray(np.asarray(inputs["bae1"], f32).reshape(128, 1)),
        "b_ae2": np.ascontiguousarray(np.asarray(inputs["bae2"], f32).reshape(AGG_E, 1)),
        "b_le1": np.ascontiguousarray(np.asarray(inputs["ble1"], f32).reshape(96, 1)),
        "b_le2": np.ascontiguousarray(np.asarray(inputs["ble2"], f32).reshape(OUT_E, 1)),
        "b_ln1a": np.ascontiguousarray(np.asarray(inputs["bln1"], f32)[0:128].reshape(128, 1)),
        "b_ln1b": np.ascontiguousarray(np.asarray(inputs["bln1"], f32)[128:192].reshape(64, 1)),
        "b_ln2": np.ascontiguousarray(np.asarray(inputs["bln2"], f32).reshape(OUT_N, 1)),
    }
    return m


def kernel(**inputs):
    import sys
    if "/opt/trn_rl_repo" not in sys.path:
        sys.path.insert(0, "/opt/trn_rl_repo")
    from concourse.bass_utils import run_bass_kernel_spmd

    zb = all(not np.any(np.asarray(inputs[k])) for k in
             ("ban1", "ban2", "bae1", "bae2", "ble1", "ble2", "bln1", "bln2"))
    nc = _build_program(zero_bias=zb)
    in_maps = []
    for c in range(NCORES):
        b, q = c // NQ, c % NQ
        in_maps.append(_prep_core_inputs(inputs, b, q))
    res = run_bass_kernel_spmd(nc, in_maps, list(range(NCORES))).results

    valid = _host_tables()[0]
    out_nodes = np.zeros((B, N, OUT_N), np.float32)
    out_edges = np.zeros((B, NE, OUT_E), np.float32)
    for c in range(NCORES):
        b, q = c // NQ, c % NQ
        n0 = q * NSH
        out_nodes[b, n0:n0 + NSH] = res[c]["out_nodes"]
        if q == 0:
            out_edges[b, 0:_koff(NSH)] = res[c]["out_edges"][valid]
        else:
            out_edges[b, _koff(n0):_koff(n0) + ES] = res[c]["out_edges"]
    return out_nodes, out_edges
